# revision 2
# baseline (speedup 1.0000x reference)
"""MultiHeadAttention+RoPE Trainium2 kernel, 8-way sharded.

Sharding: core c handles batch b=c//2 and head-half hh=c%2 (8 of 16 heads).
Per core: q/k/v projections for its heads (transposed GEMMs, fp32r), RoPE,
attention with transposed scores [keys, q], exp on ScalarE (scale folded in),
P*V in bf16 with a ones-column for the softmax denominator, per-head
normalization, and the fc projection over its 512 head-dims. The two cores of
each batch produce partial fc outputs that the host sums (no collectives).

Expected full shapes (hardcoded): q/k/v [4,2048,1024] f32,
rope_cos/rope_sin [1,1,2048,64], w_qs/w_ks/w_vs [1024,1024], w_fc [1024,1024].
Returns [4,2048,1024] f32.
"""
import sys

for p in ("/opt/trn_rl_repo", "/root/.axon_site", "/root/.axon_site/_ro/trn_rl_repo"):
    if p not in sys.path:
        sys.path.append(p)

import numpy as np

B, S, D, H, DK, DV = 4, 2048, 1024, 16, 64, 64
HC = H // 2          # heads per core
HP = HC // 2         # head-pairs per core
NCORES = 8
QC = 512             # query chunk (matmul free dim)
NQC = S // QC        # 4
KB = 128             # key block
NKB = S // KB        # 16
CH = HC * DK         # 512 projection channels per core

_PROGRAM = None


def _build_program():
    import concourse.tile as tile
    from concourse import bacc, mybir

    FR, F32, BF16 = mybir.dt.float32r, mybir.dt.float32, mybir.dt.bfloat16
    Exp = mybir.ActivationFunctionType.Exp

    nc = bacc.Bacc("TRN2", target_bir_lowering=False, debug=False,
                   num_devices=NCORES)

    xqT = nc.declare_dram_parameter("xqT", [D, S], FR, isOutput=False)
    xkT = nc.declare_dram_parameter("xkT", [D, S], FR, isOutput=False)
    xvT = nc.declare_dram_parameter("xvT", [D, S], FR, isOutput=False)
    wqT = nc.declare_dram_parameter("wqT", [D, CH], FR, isOutput=False)
    wkT = nc.declare_dram_parameter("wkT", [D, CH], FR, isOutput=False)
    wvT = nc.declare_dram_parameter("wvT", [D, CH], FR, isOutput=False)
    wfcT = nc.declare_dram_parameter("wfcT", [CH, D], FR, isOutput=False)
    cosP = nc.declare_dram_parameter("cosP", [128, S], F32, isOutput=False)
    sinF = nc.declare_dram_parameter("sinF", [128, S], F32, isOutput=False)
    outT = nc.declare_dram_parameter("outT", [D, S], F32, isOutput=True)

    ND = D // 128  # 8 contraction chunks

    with tile.TileContext(nc) as tc:
        with (
            tc.tile_pool(name="persist", bufs=1) as pp,
            tc.tile_pool(name="psum", bufs=2, space="PSUM") as ps,
        ):
            cos_sb = pp.tile([128, S], F32, name="cos_sb", tag="cos")
            sin_sb = pp.tile([128, S], F32, name="sin_sb", tag="sin")
            nc.sync.dma_start(cos_sb[:], cosP[:])
            nc.sync.dma_start(sin_sb[:], sinF[:])

            qh = [pp.tile([128, S], FR, name=f"qh{m}", tag=f"qh{m}")
                  for m in range(HP)]
            kh = [pp.tile([128, S], FR, name=f"kh{m}", tag=f"kh{m}")
                  for m in range(HP)]
            vh = pp.tile([128, NKB, HC * (DV + 1)], BF16, name="vh", tag="vh")

            # ---------------- projections ----------------
            with (
                tc.tile_pool(name="projw", bufs=1) as pw,
                tc.tile_pool(name="xload", bufs=3) as xl,
                tc.tile_pool(name="ropes", bufs=3) as rp,
            ):
                wq_sb = pw.tile([128, ND, CH], FR, name="wq_sb", tag="wq")
                wk_sb = pw.tile([128, ND, CH], FR, name="wk_sb", tag="wk")
                wv_sb = pw.tile([128, ND, CH], FR, name="wv_sb", tag="wv")
                for d in range(ND):
                    nc.sync.dma_start(wq_sb[:, d, :], wqT[d * 128:(d + 1) * 128, :])
                    nc.sync.dma_start(wk_sb[:, d, :], wkT[d * 128:(d + 1) * 128, :])
                    nc.sync.dma_start(wv_sb[:, d, :], wvT[d * 128:(d + 1) * 128, :])

                # Q and K projections -> transposed layout [ch, rows] + RoPE
                for xT, w_sb, dst in ((xqT, wq_sb, qh), (xkT, wk_sb, kh)):
                    for rc in range(NQC):
                        pm0 = ps.tile([128, 2, QC], F32, name="pm0", tag="sc")
                        pm1 = ps.tile([128, 2, QC], F32, name="pm1", tag="sc")
                        pms = (pm0[:, 0, :], pm0[:, 1, :], pm1[:, 0, :], pm1[:, 1, :])
                        for d in range(ND):
                            xt = xl.tile([128, QC], FR, name="xt", tag="x")
                            nc.sync.dma_start(
                                xt[:], xT[d * 128:(d + 1) * 128,
                                          rc * QC:(rc + 1) * QC])
                            for m in range(HP):
                                nc.tensor.matmul(
                                    pms[m], w_sb[:, d, m * 128:(m + 1) * 128],
                                    xt[:], start=(d == 0), stop=(d == ND - 1))
                        for m in range(HP):
                            raw = rp.tile([128, QC], F32, name="raw", tag="raw")
                            nc.scalar.copy(raw[:], pms[m])
                            swp = rp.tile([128, QC], F32, name="swp", tag="swp")
                            for blk in range(4):
                                b0, b1 = blk * 32, (blk ^ 1) * 32
                                nc.sync.dma_start(swp[b0:b0 + 32, :],
                                                  raw[b1:b1 + 32, :])
                            t1 = rp.tile([128, QC], F32, name="t1", tag="t1")
                            nc.vector.tensor_mul(
                                t1[:], pms[m], cos_sb[:, rc * QC:(rc + 1) * QC])
                            t2 = rp.tile([128, QC], F32, name="t2", tag="t2")
                            nc.vector.tensor_mul(
                                t2[:], swp[:], sin_sb[:, rc * QC:(rc + 1) * QC])
                            nc.vector.tensor_add(
                                dst[m][:, rc * QC:(rc + 1) * QC], t1[:], t2[:])

                # V projection -> [rows, ch] with ones columns
                vh_r = vh.rearrange("p r (h x) -> p r h x", h=HC)
                nc.vector.memset(vh_r[:, :, :, DV], 1.0)
                for rg in range(4):  # groups of 4 row-blocks
                    pv0 = ps.tile([128, 2, QC], F32, name="pv0", tag="sc")
                    pv1 = ps.tile([128, 2, QC], F32, name="pv1", tag="sc")
                    pvs = (pv0[:, 0, :], pv0[:, 1, :], pv1[:, 0, :], pv1[:, 1, :])
                    for d in range(ND):
                        xt = xl.tile([128, QC], FR, name="xt", tag="x")
                        nc.sync.dma_start(
                            xt[:], xvT[d * 128:(d + 1) * 128,
                                       rg * 512:(rg + 1) * 512])
                        for j in range(4):
                            nc.tensor.matmul(
                                pvs[j], xt[:, j * 128:(j + 1) * 128],
                                wv_sb[:, d, :], start=(d == 0), stop=(d == ND - 1))
                    for j in range(4):
                        rb = rg * 4 + j
                        nc.vector.tensor_copy(
                            vh_r[:, rb, :, 0:DV],
                            pvs[j].rearrange("p (h x) -> p h x", h=HC))

            # ---------------- attention + fc ----------------
            with (
                tc.tile_pool(name="wfcp", bufs=1) as wp,
                tc.tile_pool(name="xsp", bufs=2) as xs,
                tc.tile_pool(name="frh", bufs=2) as fr_,
                tc.tile_pool(name="nrm", bufs=4) as nm,
                tc.tile_pool(name="oeva", bufs=3) as oe,
            ):
                wfc_sb = [wp.tile([128, D], FR, name=f"wfc{i}", tag=f"wfc{i}")
                          for i in range(HP)]
                for i in range(HP):
                    nc.sync.dma_start(wfc_sb[i][:], wfcT[i * 128:(i + 1) * 128, :])

                for qc in range(NQC):
                    fcrhs = fr_.tile([128, HP, QC], FR, name="fcrhs", tag="fcrhs")
                    for hp in range(HP):
                        xpA = xs.tile([128, NKB, QC], BF16, name="xpA", tag="xp")
                        xpB = xs.tile([128, NKB, QC], BF16, name="xpB", tag="xp")
                        qsl = slice(qc * QC, (qc + 1) * QC)
                        for kb2 in range(NKB // 2):
                            scA = ps.tile([128, 2, QC], F32, name="scA", tag="sc")
                            scB = ps.tile([128, 2, QC], F32, name="scB", tag="sc")
                            for i in range(2):
                                kb = kb2 * 2 + i
                                ksl = slice(kb * KB, (kb + 1) * KB)
                                nc.tensor.matmul(scA[:, i, :], kh[hp][0:64, ksl],
                                                 qh[hp][0:64, qsl])
                                nc.tensor.matmul(scB[:, i, :], kh[hp][64:128, ksl],
                                                 qh[hp][64:128, qsl])
                            sc2 = slice(kb2 * 2, kb2 * 2 + 2)
                            nc.scalar.activation(xpA[:, sc2, :], scA[:],
                                                 Exp, scale=DK ** -0.5)
                            nc.scalar.activation(xpB[:, sc2, :], scB[:],
                                                 Exp, scale=DK ** -0.5)
                        for jl, xp in ((0, xpA), (1, xpB)):
                            j = hp * 2 + jl
                            ctx = ps.tile([DV + 1, QC], F32, name="ctx", tag="ctx")
                            for kb in range(NKB):
                                nc.tensor.matmul(
                                    ctx[:],
                                    vh[:, kb, j * (DV + 1):(j + 1) * (DV + 1)],
                                    xp[:, kb, :],
                                    start=(kb == 0), stop=(kb == NKB - 1))
                            rr = nm.tile([1, QC], F32, name="rr", tag="rr")
                            nc.vector.reciprocal(rr[:], ctx[DV:DV + 1, :])
                            bc = nm.tile([DV, QC], F32, name="bc", tag="bc")
                            nc.gpsimd.partition_broadcast(bc[:], rr[:])
                            nc.vector.tensor_mul(
                                fcrhs[jl * 64:jl * 64 + 64, hp, :],
                                ctx[0:DV, :], bc[:])
                    for ob in range(D // 128):
                        fo = ps.tile([128, QC], F32, name="fo", tag="fo")
                        for hp in range(HP):
                            nc.tensor.matmul(
                                fo[:], wfc_sb[hp][:, ob * 128:(ob + 1) * 128],
                                fcrhs[:, hp, :],
                                start=(hp == 0), stop=(hp == HP - 1))
                        osb = oe.tile([128, QC], F32, name="osb", tag="osb")
                        nc.vector.tensor_copy(osb[:], fo[:])
                        nc.sync.dma_start(
                            outT[ob * 128:(ob + 1) * 128, qc * QC:(qc + 1) * QC],
                            osb[:])

    nc.compile()
    return nc


def _host_prep(q, k, v, rope_cos, rope_sin, w_qs, w_ks, w_vs, w_fc):
    f32 = np.float32
    cosT = np.ascontiguousarray(rope_cos.reshape(S, DK).T, dtype=f32)
    sinT = np.ascontiguousarray(rope_sin.reshape(S, DK).T, dtype=f32)
    cosP = np.concatenate([cosT, cosT], axis=0)
    sF = np.concatenate([sinT, sinT], axis=0)
    sF[0:32] *= -1.0
    sF[64:96] *= -1.0
    sF = np.ascontiguousarray(sF)

    xT = {}
    for b in range(B):
        xT[b] = (
            np.ascontiguousarray(np.asarray(q[b], dtype=f32).T),
            np.ascontiguousarray(np.asarray(k[b], dtype=f32).T),
            np.ascontiguousarray(np.asarray(v[b], dtype=f32).T),
        )
    whalf = {}
    for hh in range(2):
        rows = slice(hh * CH, (hh + 1) * CH)
        whalf[hh] = (
            np.ascontiguousarray(np.asarray(w_qs, dtype=f32)[rows].T),
            np.ascontiguousarray(np.asarray(w_ks, dtype=f32)[rows].T),
            np.ascontiguousarray(np.asarray(w_vs, dtype=f32)[rows].T),
            np.ascontiguousarray(np.asarray(w_fc, dtype=f32)[:, rows].T),
        )

    in_maps = []
    for c in range(NCORES):
        b, hh = c // 2, c % 2
        qT, kT, vT = xT[b]
        wq, wk, wv, wf = whalf[hh]
        in_maps.append({
            "xqT": qT, "xkT": kT, "xvT": vT,
            "wqT": wq, "wkT": wk, "wvT": wv, "wfcT": wf,
            "cosP": cosP, "sinF": sF,
        })
    return in_maps


def get_program():
    global _PROGRAM
    if _PROGRAM is None:
        _PROGRAM = _build_program()
    return _PROGRAM


def kernel(q, k, v, rope_cos, rope_sin, w_qs, w_ks, w_vs, w_fc):
    from concourse.bass_utils import run_bass_kernel_spmd

    nc = get_program()
    in_maps = _host_prep(q, k, v, rope_cos, rope_sin, w_qs, w_ks, w_vs, w_fc)
    res = run_bass_kernel_spmd(nc, in_maps, list(range(NCORES)))
    out = np.empty((B, S, D), dtype=np.float32)
    for b in range(B):
        acc = res.results[2 * b]["outT"] + res.results[2 * b + 1]["outT"]
        out[b] = acc.T
    return out


# revision 4
# speedup vs baseline: 1.3926x; 1.3926x over previous
"""MultiHeadAttention+RoPE Trainium2 kernel, 8-way sharded.

Sharding: core c handles batch b=c//2 and head-half hh=c%2 (8 of 16 heads).
Per core: q/k/v projections for its heads (transposed GEMMs, fp32r), RoPE,
attention with transposed scores [keys, q] in fp16 (two heads row-tiled per
score step), exp on ScalarE straight from PSUM (scale folded in, fp16 2x),
P*V in fp16 with a ones-column on V giving the softmax denominator, per-head
normalization via fast-reciprocal + gpsimd partition broadcast, and the fc
projection over this core's 512 head-dims. The attention loop is software-
pipelined: P*V of the previous head-pair is interleaved into the current
head-pair's score stream so the PE never idles. The two cores of each batch
produce partial fc outputs that the host sums (no collectives).

Expected full shapes (hardcoded): q/k/v [4,2048,1024] f32,
rope_cos/rope_sin [1,1,2048,64], w_qs/w_ks/w_vs [1024,1024], w_fc [1024,1024].
Returns [4,2048,1024] f32.
"""
import sys

for p in ("/opt/trn_rl_repo", "/root/.axon_site", "/root/.axon_site/_ro/trn_rl_repo"):
    if p not in sys.path:
        sys.path.append(p)

import numpy as np

B, S, D, H, DK, DV = 4, 2048, 1024, 16, 64, 64
HC = H // 2          # heads per core
HP = HC // 2         # head-pairs per core
NCORES = 8
QC = 512             # query chunk (matmul free dim)
NQC = S // QC        # 4
KB = 128             # key block
NKB = S // KB        # 16
CH = HC * DK         # 512 projection channels per core

_PROGRAM = None


def _build_program():
    import concourse.tile as tile
    from concourse import bacc, mybir

    FR, F32, F16 = mybir.dt.float32r, mybir.dt.float32, mybir.dt.float16
    Exp = mybir.ActivationFunctionType.Exp

    nc = bacc.Bacc("TRN2", target_bir_lowering=False, debug=False,
                   num_devices=NCORES)

    xqT = nc.declare_dram_parameter("xqT", [D, S], FR, isOutput=False)
    xkT = nc.declare_dram_parameter("xkT", [D, S], FR, isOutput=False)
    xvT = nc.declare_dram_parameter("xvT", [D, S], FR, isOutput=False)
    wqT = nc.declare_dram_parameter("wqT", [D, CH], FR, isOutput=False)
    wkT = nc.declare_dram_parameter("wkT", [D, CH], FR, isOutput=False)
    wvT = nc.declare_dram_parameter("wvT", [D, CH], FR, isOutput=False)
    wfcT = nc.declare_dram_parameter("wfcT", [CH, D], FR, isOutput=False)
    cosP = nc.declare_dram_parameter("cosP", [128, S], F32, isOutput=False)
    sinF = nc.declare_dram_parameter("sinF", [128, S], F32, isOutput=False)
    outT = nc.declare_dram_parameter("outT", [D, S], F32, isOutput=True)

    ND = D // 128  # 8 contraction chunks

    with tile.TileContext(nc) as tc:
        with (
            tc.tile_pool(name="persist", bufs=1) as pp,
            tc.tile_pool(name="psum", bufs=2, space="PSUM") as ps,
        ):
            cos_sb = pp.tile([128, S], F32, name="cos_sb", tag="cos")
            sin_sb = pp.tile([128, S], F32, name="sin_sb", tag="sin")
            nc.sync.dma_start(cos_sb[:], cosP[:])
            nc.sync.dma_start(sin_sb[:], sinF[:])

            qh = [pp.tile([128, S], F16, name=f"qh{m}", tag=f"qh{m}")
                  for m in range(HP)]
            kh = [pp.tile([128, S], F16, name=f"kh{m}", tag=f"kh{m}")
                  for m in range(HP)]
            vh = pp.tile([128, NKB, HC * (DV + 1)], F16, name="vh", tag="vh")

            def b2tile(name):
                return ps.tile([128, 2, QC], F32, name=name, tag="b2", bufs=2)

            # ---------------- projections ----------------
            with (
                tc.tile_pool(name="projw", bufs=1) as pw,
                tc.tile_pool(name="xload", bufs=3) as xl,
                tc.tile_pool(name="ropes", bufs=3) as rp,
            ):
                wq_sb = pw.tile([128, ND, CH], FR, name="wq_sb", tag="wq")
                wk_sb = pw.tile([128, ND, CH], FR, name="wk_sb", tag="wk")
                wv_sb = pw.tile([128, ND, CH], FR, name="wv_sb", tag="wv")
                for d in range(ND):
                    nc.sync.dma_start(wq_sb[:, d, :], wqT[d * 128:(d + 1) * 128, :])
                    nc.sync.dma_start(wk_sb[:, d, :], wkT[d * 128:(d + 1) * 128, :])
                    nc.sync.dma_start(wv_sb[:, d, :], wvT[d * 128:(d + 1) * 128, :])

                # Q and K projections -> transposed layout [ch, rows] + RoPE
                for xT, w_sb, dst in ((xqT, wq_sb, qh), (xkT, wk_sb, kh)):
                    for rc in range(NQC):
                        pm0 = b2tile("pm0")
                        pm1 = b2tile("pm1")
                        pms = (pm0[:, 0, :], pm0[:, 1, :], pm1[:, 0, :], pm1[:, 1, :])
                        for d in range(ND):
                            xt = xl.tile([128, QC], FR, name="xt", tag="x")
                            nc.sync.dma_start(
                                xt[:], xT[d * 128:(d + 1) * 128,
                                          rc * QC:(rc + 1) * QC])
                            for m in range(HP):
                                nc.tensor.matmul(
                                    pms[m], w_sb[:, d, m * 128:(m + 1) * 128],
                                    xt[:], start=(d == 0), stop=(d == ND - 1))
                        for m in range(HP):
                            raw = rp.tile([128, QC], F32, name="raw", tag="raw")
                            nc.scalar.copy(raw[:], pms[m])
                            swp = rp.tile([128, QC], F32, name="swp", tag="swp")
                            for blk in range(4):
                                b0, b1 = blk * 32, (blk ^ 1) * 32
                                nc.sync.dma_start(swp[b0:b0 + 32, :],
                                                  raw[b1:b1 + 32, :])
                            t1 = rp.tile([128, QC], F32, name="t1", tag="t1")
                            nc.vector.tensor_mul(
                                t1[:], pms[m], cos_sb[:, rc * QC:(rc + 1) * QC])
                            t2 = rp.tile([128, QC], F32, name="t2", tag="t2")
                            nc.vector.tensor_mul(
                                t2[:], swp[:], sin_sb[:, rc * QC:(rc + 1) * QC])
                            nc.vector.tensor_add(
                                dst[m][:, rc * QC:(rc + 1) * QC], t1[:], t2[:])

                # V projection -> [rows, ch] with ones columns
                vh_r = vh.rearrange("p r (h x) -> p r h x", h=HC)
                nc.vector.memset(vh_r[:, :, :, DV], 1.0)
                for rg in range(4):  # groups of 4 row-blocks
                    pv0 = b2tile("pv0")
                    pv1 = b2tile("pv1")
                    pvs = (pv0[:, 0, :], pv0[:, 1, :], pv1[:, 0, :], pv1[:, 1, :])
                    for d in range(ND):
                        xt = xl.tile([128, QC], FR, name="xt", tag="x")
                        nc.sync.dma_start(
                            xt[:], xvT[d * 128:(d + 1) * 128,
                                       rg * 512:(rg + 1) * 512])
                        for j in range(4):
                            nc.tensor.matmul(
                                pvs[j], xt[:, j * 128:(j + 1) * 128],
                                wv_sb[:, d, :], start=(d == 0), stop=(d == ND - 1))
                    for j in range(4):
                        rb = rg * 4 + j
                        nc.vector.tensor_copy(
                            vh_r[:, rb, :, 0:DV],
                            pvs[j].rearrange("p (h x) -> p h x", h=HC))

            # ---------------- attention + fc (software pipelined) --------
            with (
                tc.tile_pool(name="wfcp", bufs=1) as wp,
                tc.tile_pool(name="xsp", bufs=4) as xs,
                tc.tile_pool(name="frh", bufs=2) as fr_,
                tc.tile_pool(name="nrm", bufs=4) as nm,
                tc.tile_pool(name="oeva", bufs=3) as oe,
            ):
                wfc_sb = [wp.tile([128, D], FR, name=f"wfc{i}", tag=f"wfc{i}")
                          for i in range(HP)]
                for i in range(HP):
                    nc.sync.dma_start(wfc_sb[i][:], wfcT[i * 128:(i + 1) * 128, :])

                steps = [(qc, hp) for qc in range(NQC) for hp in range(HP)]
                fcrhs_by_qc = {}
                prev = None  # (qc, hp, xpA, xpB)

                def issue_pv_slot(state, kb2, ctx):
                    _, php, xpa, xpb = state
                    for jl, xp in ((0, xpa), (1, xpb)):
                        j = php * 2 + jl
                        for i in range(2):
                            kb = kb2 * 2 + i
                            nc.tensor.matmul(
                                ctx[0:DV + 1, jl, :],
                                vh[:, kb, j * (DV + 1):(j + 1) * (DV + 1)],
                                xp[:, kb, :],
                                start=(kb == 0), stop=(kb == NKB - 1))

                def issue_norm(state, ctx):
                    pqc, php, _, _ = state
                    fcr = fcrhs_by_qc[pqc]
                    for jl in range(2):
                        rr = nm.tile([1, QC], F32, name="rr", tag="rr")
                        nc.vector.reciprocal_approx_fast(
                            rr[:], ctx[DV:DV + 1, jl, :])
                        bc = nm.tile([DV, QC], F32, name="bc", tag="bc")
                        nc.gpsimd.partition_broadcast(bc[:], rr[:])
                        nc.vector.tensor_mul(
                            fcr[jl * 64:jl * 64 + 64, php, :],
                            ctx[0:DV, jl, :], bc[:])

                def issue_fc(pqc):
                    fcr = fcrhs_by_qc.pop(pqc)
                    for obp in range(4):
                        fo = b2tile("fo")
                        for i in range(2):
                            ob = obp * 2 + i
                            for hp_ in range(HP):
                                nc.tensor.matmul(
                                    fo[:, i, :],
                                    wfc_sb[hp_][:, ob * 128:(ob + 1) * 128],
                                    fcr[:, hp_, :],
                                    start=(hp_ == 0), stop=(hp_ == HP - 1))
                        osb = oe.tile([128, 2, QC], F32, name="osb", tag="osb")
                        nc.vector.tensor_copy(osb[:], fo[:])
                        dst = outT[obp * 256:(obp + 1) * 256,
                                   pqc * QC:(pqc + 1) * QC]
                        nc.sync.dma_start(
                            dst.rearrange("(t p) n -> p t n", p=128), osb[:])

                for step in range(len(steps) + 1):
                    cur = steps[step] if step < len(steps) else None
                    ctx = b2tile("ctx") if prev else None
                    if cur:
                        qc, hp = cur
                        if hp == 0:
                            fcrhs_by_qc[qc] = fr_.tile(
                                [128, HP, QC], FR, name="fcrhs", tag="fcrhs")
                        qsl = slice(qc * QC, (qc + 1) * QC)
                        xpA = xs.tile([128, NKB, QC], F16, name="xpA", tag="xp")
                        xpB = xs.tile([128, NKB, QC], F16, name="xpB", tag="xp")
                        for kb2 in range(NKB // 2):
                            scA = ps.tile([128, 2, QC], F32, name="scA", tag="sc")
                            scB = ps.tile([128, 2, QC], F32, name="scB", tag="sc")
                            for i in range(2):
                                kb = kb2 * 2 + i
                                ksl = slice(kb * KB, (kb + 1) * KB)
                                nc.tensor.matmul(scA[:, i, :], kh[hp][0:64, ksl],
                                                 qh[hp][0:64, qsl])
                                nc.tensor.matmul(scB[:, i, :], kh[hp][64:128, ksl],
                                                 qh[hp][64:128, qsl])
                            sc2 = slice(kb2 * 2, kb2 * 2 + 2)
                            nc.scalar.activation(xpA[:, sc2, :], scA[:],
                                                 Exp, scale=DK ** -0.5)
                            nc.scalar.activation(xpB[:, sc2, :], scB[:],
                                                 Exp, scale=DK ** -0.5)
                            if prev:
                                issue_pv_slot(prev, kb2, ctx)
                    else:
                        for kb2 in range(NKB // 2):
                            issue_pv_slot(prev, kb2, ctx)
                    if prev:
                        issue_norm(prev, ctx)
                        if prev[1] == HP - 1:
                            issue_fc(prev[0])
                    prev = (cur[0], cur[1], xpA, xpB) if cur else None

    nc.compile()
    return nc


def _host_prep(q, k, v, rope_cos, rope_sin, w_qs, w_ks, w_vs, w_fc):
    f32 = np.float32
    cosT = np.ascontiguousarray(rope_cos.reshape(S, DK).T, dtype=f32)
    sinT = np.ascontiguousarray(rope_sin.reshape(S, DK).T, dtype=f32)
    cosP = np.concatenate([cosT, cosT], axis=0)
    sF = np.concatenate([sinT, sinT], axis=0)
    sF[0:32] *= -1.0
    sF[64:96] *= -1.0
    sF = np.ascontiguousarray(sF)

    xT = {}
    for b in range(B):
        xT[b] = (
            np.ascontiguousarray(np.asarray(q[b], dtype=f32).T),
            np.ascontiguousarray(np.asarray(k[b], dtype=f32).T),
            np.ascontiguousarray(np.asarray(v[b], dtype=f32).T),
        )
    whalf = {}
    for hh in range(2):
        rows = slice(hh * CH, (hh + 1) * CH)
        whalf[hh] = (
            np.ascontiguousarray(np.asarray(w_qs, dtype=f32)[rows].T),
            np.ascontiguousarray(np.asarray(w_ks, dtype=f32)[rows].T),
            np.ascontiguousarray(np.asarray(w_vs, dtype=f32)[rows].T),
            np.ascontiguousarray(np.asarray(w_fc, dtype=f32)[:, rows].T),
        )

    in_maps = []
    for c in range(NCORES):
        b, hh = c // 2, c % 2
        qT, kT, vT = xT[b]
        wq, wk, wv, wf = whalf[hh]
        in_maps.append({
            "xqT": qT, "xkT": kT, "xvT": vT,
            "wqT": wq, "wkT": wk, "wvT": wv, "wfcT": wf,
            "cosP": cosP, "sinF": sF,
        })
    return in_maps


def get_program():
    global _PROGRAM
    if _PROGRAM is None:
        _PROGRAM = _build_program()
    return _PROGRAM


def kernel(q, k, v, rope_cos, rope_sin, w_qs, w_ks, w_vs, w_fc):
    from concourse.bass_utils import run_bass_kernel_spmd

    nc = get_program()
    in_maps = _host_prep(q, k, v, rope_cos, rope_sin, w_qs, w_ks, w_vs, w_fc)
    res = run_bass_kernel_spmd(nc, in_maps, list(range(NCORES)))
    out = np.empty((B, S, D), dtype=np.float32)
    for b in range(B):
        acc = res.results[2 * b]["outT"] + res.results[2 * b + 1]["outT"]
        out[b] = acc.T
    return out
